# revision 31
# baseline (speedup 1.0000x reference)
"""Trainium2 Bass kernel for nn_MaskedAttention (B=2, N=2048, C=1024, H=16).

Sharding: batch x head-group over 8 cores (core c -> batch c//4, heads
4*(c%4)..4*(c%4)+3).  The reference's "faithful" head-scrambled reshape
means each head's output occupies a contiguous 128-row block of the
pre-projection matrix, so the output projection is row-parallel across
heads and needs no cross-core reduction.

Pipeline highlights (vs the straightforward version):
  - all matmul operands bf16 (halves input DMA, enables fast weight load);
    fp32 only in PSUM accumulators and the softmax denominator path.
  - k-bias dropped (softmax-invariant: it shifts each score row by a
    per-row constant); v-bias folded into an effective projection bias on
    the host (bp_eff = b_proj + w_proj @ tile(bv_h)); only q keeps its bias.
  - scores computed transposed sT[j,i] per 128x512 tile for both heads of
    a pair at once (row groups 0/64 -> concurrent on HW); exp reads the
    two heads' tiles as one [128,1024] PSUM-spanning activation.
  - AV uses the augmented-[V|1] stationary trick: row 64 of the PSUM
    output is the softmax denominator for free.
  - reciprocal on DVE, partition-broadcast on GPSIMD (Pool), diagonal-tile
    memsets on Pool; normalization multiply writes directly in the
    head-scrambled projection staging layout (no separate copies).
  - QKV / V / projection chains are interleaved into the attention stream
    as PE "filler" work so the PE never idles while ACT grinds exp.
"""

import numpy as np

import concourse.bass as bass
import concourse.mybir as mybir
from concourse import tile
from concourse.bass_utils import run_bass_kernel_spmd

B, N, C, H = 2, 2048, 1024, 16
D = C // H                 # 64
SCALE = D ** -0.5
EBIAS = -20.0
P = 128
NB = N // 512              # 4 n blocks
NJT = N // P               # 16 j tiles
F32 = mybir.dt.float32
BF16 = mybir.dt.bfloat16
AF = mybir.ActivationFunctionType

# Col-pack the two heads of a pair into one AV matmul round (tile_position
# (0,0)/(0,64)) with 4-way-concurrent M=1 denominator matmuls.  Dead end:
# each PSUM accumulation chain needs exclusive ownership of its bank's
# zero region (start=True clears the bank), so the packed layout needs 9
# banks against the 8 available.  Kept for documentation.
AV_PACK = False


def _emit(nc: bass.Bass, d: dict, repeats: int = 1):
    from contextlib import ExitStack

    with tile.TileContext(nc) as tc, ExitStack() as ctx:
        const = ctx.enter_context(tc.tile_pool(name="const", bufs=1))
        wqk = const.tile([P, 8, 512], BF16)
        wv = const.tile([P, 8, 256], BF16)
        bq = const.tile([P, 2], F32)
        tri2 = const.tile([P, 2, P], BF16)
        ebias = const.tile([P, 1], F32)
        qk = const.tile([P, 4, N], BF16)         # [p, {qq0,qq1,kk0,kk1}, n]
        xT = const.tile([P, 8, N], BF16)
        vaug = const.tile([P, NJT, 4 * 65], BF16)
        wp = const.tile([P, 8, 1024], BF16)
        bp = const.tile([P, 4, 1024], BF16)      # per-head effective proj bias

        nc.vector.memset(ebias[:], EBIAS)
        nc.gpsimd.memset(vaug[:], 1.0)
        ones1 = const.tile([P, 1], BF16)
        nc.vector.memset(ones1[:], 1.0)
        onesf = const.tile([1, 64], F32)
        onesr = const.tile([1, 64], mybir.dt.float32r)
        nc.vector.memset(onesf[:], 1.0)
        nc.vector.tensor_copy(onesr[:], onesf[:])
        # pre-warm the exp table set while the first DMAs are in flight
        warm = const.tile([P, 1], F32)
        nc.scalar.activation(warm[:], ebias[:], AF.Exp)

        for _rep in range(repeats):
            with tc.tile_pool(name="sps", bufs=2, space="PSUM") as sps, \
                 tc.tile_pool(name="avps", bufs=1, space="PSUM") as avps, \
                 tc.tile_pool(name="fps", bufs=2, space="PSUM") as fps, \
                 tc.tile_pool(name="att", bufs=2) as att, \
                 tc.tile_pool(name="stg", bufs=2) as stg, \
                 tc.tile_pool(name="post", bufs=3) as post:
                # ---------- input DMAs ----------
                nc.sync.dma_start(
                    wqk[:, 0:2, :], d["w_qk"][0:2].rearrange("c p w -> p c w"))
                nc.sync.dma_start(
                    xT[:, 0:2, 0:512],
                    d["xT"][0, 0:2].rearrange("c p w -> p c w"))
                nc.sync.dma_start(
                    wqk[:, 2:8, :], d["w_qk"][2:8].rearrange("c p w -> p c w"))
                nc.sync.dma_start(
                    xT[:, 2:8, 0:512],
                    d["xT"][0, 2:8].rearrange("c p w -> p c w"))
                nc.sync.dma_start(bq[:], d["b_q"][:])
                nc.sync.dma_start(wv[:], d["w_v"].rearrange("c p w -> p c w"))
                for nb in range(1, NB):
                    nc.sync.dma_start(
                        xT[:, :, 512 * nb:512 * nb + 512],
                        d["xT"][nb].rearrange("c p w -> p c w"))
                nc.sync.dma_start(tri2[:], d["tri2"][:])
                nc.sync.dma_start(wp[:], d["w_p"].rearrange("c p w -> p c w"))
                nc.sync.dma_start(bp[:], d["b_p"].rearrange("h p w -> p h w"))

                # ---------- PE work-chunk emitters ----------
                def emit_qk_chain(mb, nb):
                    # q rows (mb 0/1) carry the bias; k rows (mb 2/3) don't
                    ps = fps.tile([P, 512], F32, tag="f")
                    for cc in range(8):
                        nc.tensor.matmul(
                            ps[:],
                            wqk[:, cc, P * mb:P * mb + P],
                            xT[:, cc, 512 * nb:512 * nb + 512],
                            start=(cc == 0), stop=(cc == 7),
                        )
                    dst = qk[:, mb, 512 * nb:512 * nb + 512]
                    if mb < 2:
                        nc.vector.tensor_scalar_add(dst, ps[:], bq[:, mb:mb + 1])
                    else:
                        nc.vector.tensor_copy(dst, ps[:])

                def emit_v_chain(jt):
                    ps = fps.tile([P, 512], F32, tag="f")
                    for cc in range(8):
                        nc.tensor.matmul(
                            ps[:, 0:256],
                            xT[:, cc, P * jt:P * jt + P],
                            wv[:, cc, :],
                            start=(cc == 0), stop=(cc == 7),
                        )
                    nc.vector.tensor_copy(
                        out=vaug[:, jt, :].rearrange(
                            "p (h x) -> p h x", x=65)[:, :, 0:64],
                        in_=ps[:, 0:256].rearrange("p (h x) -> p h x", x=64),
                    )

                def emit_proj_chunk(pair, hp, mb2, stage, osb):
                    h = 2 * pair + hp
                    ps = fps.tile([P, 512], F32, tag="f")
                    for k in range(8):
                        nc.tensor.matmul(
                            ps[:],
                            stage[:, k, :],
                            wp[:, k, 512 * mb2:512 * mb2 + 512],
                            start=(k == 0), stop=(k == 7),
                        )
                    nc.vector.tensor_add(
                        out=osb[:, 512 * mb2:512 * mb2 + 512], in0=ps[:],
                        in1=bp[:, h, 512 * mb2:512 * mb2 + 512])
                    nc.gpsimd.dma_start(
                        d["out"][P * h:P * h + P, 512 * mb2:512 * mb2 + 512],
                        osb[:, 512 * mb2:512 * mb2 + 512])

                # ---------- prologue: q/k for pair 0, v for jt 0-3 ----------
                for nb in range(NB):
                    emit_qk_chain(0, nb)
                    emit_qk_chain(2, nb)
                for jt in range(4):
                    emit_v_chain(jt)

                # ---------- attention with deadline-woven PE fillers ------
                proj_state = {}

                def run_filler(f):
                    if f[0] == "v":
                        emit_v_chain(f[1])
                    elif f[0] == "qk":
                        emit_qk_chain(f[1], f[2])
                    else:
                        _, pair, hp, mb2 = f
                        key = (pair, hp)
                        if key not in proj_state:
                            proj_state[key] = post.tile([P, 1024], F32, tag="osb",
                                                        name=f"osb{pair}{hp}")
                        emit_proj_chunk(pair, hp, mb2, stages[key],
                                        proj_state[key])

                def make_fillers(pair):
                    # ordered so force-draining to a deadline key is safe
                    fills = []
                    if pair == 0:
                        for jt in range(4, NJT):
                            fills.append(("v", jt))
                        for nb in range(NB):
                            fills.append(("qk", 1, nb))
                            fills.append(("qk", 3, nb))
                    else:
                        for hp in range(2):
                            for mb2 in range(2):
                                fills.append(("proj", 0, hp, mb2))
                    return fills

                stages = {}
                for pair in range(2):
                    fillers = make_fillers(pair)
                    fi = 0
                    nbatch = sum(4 * (m + 1) for m in range(NB))

                    def drain_until(fkey):
                        # emit fillers up to and including fkey (if pending)
                        nonlocal fi
                        if fkey not in fillers[fi:]:
                            return
                        stop = fillers.index(fkey, fi)
                        while fi <= stop:
                            run_filler(fillers[fi])
                            fi += 1
                    for hp in range(2):
                        stages[(pair, hp)] = stg.tile([P, 8, P], BF16, tag=f"stage{hp}", name=f"stage{pair}{hp}")
                    bi = 0
                    for m in range(NB):
                        njt = 4 * (m + 1)
                        expT = att.tile([P, NJT, 2, 512], BF16, tag="expT")
                        if AV_PACK:
                            av = avps.tile([P, 512], F32, tag="av0", name="av")
                            den = avps.tile([P, 512], F32, tag="av1",
                                            name="den")
                            pss = None
                        else:
                            pss = [avps.tile([65, 512], F32, tag=f"av{hp}",
                                             name=f"av{hp}")
                                   for hp in range(2)]

                        def emit_scores(jt):
                            s = sps.tile([P, 1024], F32, tag="s")
                            for hp in range(2):
                                lo = 64 * hp
                                nc.tensor.matmul(
                                    s[:, 512 * hp:512 * hp + 512],
                                    qk[lo:lo + 64, 2 + pair,
                                       P * jt:P * jt + P],
                                    qk[lo:lo + 64, pair,
                                       512 * m:512 * m + 512],
                                    start=True, stop=True,
                                )
                            t = jt - 4 * m
                            ex = expT[:, jt, :, :]
                            if t < 0:
                                nc.scalar.activation(
                                    ex.rearrange("p h w -> p (h w)"),
                                    s[:], AF.Exp, bias=ebias[:], scale=SCALE)
                            else:
                                if t > 0:
                                    nc.gpsimd.memset(ex[:, :, 0:P * t], 0.0)
                                nc.scalar.activation(
                                    ex[:, :, P * t:512],
                                    s[:].rearrange(
                                        "p (h w) -> p h w", h=2)[:, :, P * t:512],
                                    AF.Exp, bias=ebias[:], scale=SCALE)
                                nc.vector.tensor_mul(
                                    out=ex[:, :, P * t:P * t + P],
                                    in0=ex[:, :, P * t:P * t + P], in1=tri2[:])

                        def emit_av(jt):
                            if AV_PACK:
                                for hp in range(2):
                                    h = 2 * pair + hp
                                    nc.tensor.matmul(
                                        av[64 * hp:64 * hp + 64, :],
                                        vaug[:, jt, 65 * h:65 * h + 64],
                                        expT[:, jt, hp, :],
                                        start=(jt == 0), stop=(jt == njt - 1),
                                        tile_position=(0, 64 * hp),
                                    )
                                if jt % 2 == 1:
                                    # 4 concurrent M=1 denominator columns
                                    for jj in (jt - 1, jt):
                                        for hp in range(2):
                                            c = hp + 2 * (jj % 2)
                                            nc.tensor.matmul(
                                                den[32 * c:32 * c + 1, :],
                                                ones1[:],
                                                expT[:, jj, hp, :],
                                                start=(jj < 2),
                                                stop=(jj >= njt - 2),
                                                tile_position=(0, 32 * c),
                                            )
                            else:
                                for hp in range(2):
                                    h = 2 * pair + hp
                                    nc.tensor.matmul(
                                        pss[hp][:],
                                        vaug[:, jt, 65 * h:65 * h + 65],
                                        expT[:, jt, hp, :],
                                        start=(jt == 0), stop=(jt == njt - 1),
                                    )

                        # deadlines: q/k chains this m-block's scores need
                        if pair == 0 and m > 0:
                            drain_until(("qk", 2, m))
                        # software-pipelined: scores run two tiles ahead of AV
                        emit_scores(0)
                        if njt > 1:
                            emit_scores(1)
                        for jt in range(njt):
                            if jt + 2 < njt:
                                emit_scores(jt + 2)
                            if pair == 0:
                                drain_until(("v", jt))
                            emit_av(jt)
                            bi += 1
                            # weave PE filler chunks evenly across batches
                            if fi < len(fillers) and bi * len(fillers) >= \
                                    nbatch * (fi + 1):
                                run_filler(fillers[fi])
                                fi += 1
                        # normalization directly into the staged layout
                        for hp in range(2):
                            last_m = m == NB - 1
                            rec = post.tile(
                                [1, 512],
                                mybir.dt.float32r if last_m else F32,
                                tag="recr" if last_m else "rec", name="rec")
                            if AV_PACK:
                                dsum = post.tile([1, 512], F32, tag="dsum")
                                nc.vector.tensor_add(
                                    out=dsum[:],
                                    in0=den[32 * hp:32 * hp + 1, :],
                                    in1=den[64 + 32 * hp:65 + 32 * hp, :])
                                nc.vector.reciprocal(rec[:], dsum[:])
                                avsrc = av[64 * hp:64 * hp + 64, :]
                            else:
                                ps_o = pss[hp]
                                with nc.allow_low_precision(
                                        reason="f32r tag for fast bcast"):
                                    nc.vector.reciprocal(rec[:], ps_o[64:65, :])
                                avsrc = ps_o[0:64, :]
                            if last_m:
                                # tail: PE is idle here and the broadcast-DMA
                                # latency (~2us) would gate the last projs --
                                # broadcast via a K=1 matmul instead (the mul
                                # can't read two PSUM operands, so bounce the
                                # broadcast through SBUF)
                                bcp = fps.tile([64, 512], F32, tag="f",
                                               name=f"bcp{hp}")
                                nc.tensor.matmul(bcp[:], onesr[:], rec[:],
                                                 start=True, stop=True)
                                bc = post.tile([64, 512], F32, tag="bc")
                                nc.vector.tensor_copy(bc[:], bcp[:])
                            else:
                                bc = post.tile([64, 512], F32, tag="bc")
                                nc.gpsimd.dma_start(
                                    bc[:],
                                    rec[0:1, None, :].broadcast_to(
                                        (1, 64, 512)))
                            stage = stages[(pair, hp)]
                            for e in range(2):
                                nc.vector.tensor_mul(
                                    out=stage[64 * e:64 * e + 64, :,
                                              32 * m:32 * m + 32],
                                    in0=avsrc.rearrange(
                                        "p (q k e) -> p k q e",
                                        k=8, e=2)[:, :, :, e],
                                    in1=bc[:, :].rearrange(
                                        "p (q k e) -> p k q e",
                                        k=8, e=2)[:, :, :, e],
                                )
                            if pair == 1 and m == NB - 1:
                                # tail: project this head as soon as staged
                                run_filler(("proj", 1, hp, 0))
                                run_filler(("proj", 1, hp, 1))
                    while fi < len(fillers):
                        run_filler(fillers[fi])
                        fi += 1


def _fix_bir_for_walrus(bir: bytes) -> bytes:
    """Split multi-semaphore-wait instructions for walrus builds that
    support only one sync-wait command per instruction: extra waits are
    hoisted onto same-engine NoOps inserted immediately before.  ISA-class
    (custom Pool) instructions get ALL waits hoisted."""
    import json as _json

    d = _json.loads(bir)
    uid = [0]
    for fn in d["functions"]:
        for blk in fn["blocks"]:
            out = []
            for inst in blk["instructions"]:
                si = inst.get("sync_info")
                waits = (si or {}).get("on_wait") or []
                keep = 0 if "isa_opcode" in inst else 1
                if len(waits) > keep:
                    hoist, rest = waits[:len(waits) - keep], waits[len(waits) - keep:]
                    for w in hoist:
                        uid[0] += 1
                        out.append({
                            "name": f"I-wsplit-{uid[0]}",
                            "opcode": "NoOp",
                            "engine": inst["engine"],
                            "ins": [],
                            "outs": [],
                            "sync_info": {"on_wait": [w], "on_update": []},
                        })
                    si["on_wait"] = rest
                out.append(inst)
            blk["instructions"] = out
    return _json.dumps(d).encode()


_NC_CACHE = None


def build_bass(repeats: int = 1) -> bass.Bass:
    global _NC_CACHE
    if repeats == 1 and _NC_CACHE is not None:
        return _NC_CACHE
    nc = bass.Bass("TRN2", target_bir_lowering=False, debug=False,
                   enable_asserts=False, num_devices=8)
    d = {
        "xT": nc.dram_tensor("xT", [NB, 8, P, 512], BF16, kind="ExternalInput").ap(),
        "w_qk": nc.dram_tensor("w_qk", [8, P, 512], BF16, kind="ExternalInput").ap(),
        "w_v": nc.dram_tensor("w_v", [8, P, 256], BF16, kind="ExternalInput").ap(),
        "b_q": nc.dram_tensor("b_q", [P, 2], F32, kind="ExternalInput").ap(),
        "w_p": nc.dram_tensor("w_p", [8, P, 1024], BF16, kind="ExternalInput").ap(),
        "b_p": nc.dram_tensor("b_p", [4, P, 1024], BF16, kind="ExternalInput").ap(),
        "tri2": nc.dram_tensor("tri2", [P, 2, P], BF16, kind="ExternalInput").ap(),
        "out": nc.dram_tensor("out", [512, 1024], F32, kind="ExternalOutput").ap(),
    }
    _emit(nc, d, repeats=repeats)
    _orig_to_json = nc.to_json_bytes
    nc.to_json_bytes = lambda: _fix_bir_for_walrus(_orig_to_json())
    if repeats == 1:
        _NC_CACHE = nc
    return nc


def _core_inputs(core: int, x, w_qkv, b_qkv, w_proj, b_proj) -> dict:
    import ml_dtypes

    BF = ml_dtypes.bfloat16
    b = core // 4
    h0 = 4 * (core % 4)
    xT = np.ascontiguousarray(
        x[b].T.reshape(8, P, NB, 512).transpose(2, 0, 1, 3), BF)

    rows, brows = [], []
    for sec in (0, 1):                       # q section then k section
        for p in range(2):
            for e in range(2):
                h = h0 + 2 * p + e
                rows.append(w_qkv[sec * C + D * h: sec * C + D * h + D])
                brows.append(b_qkv[sec * C + D * h: sec * C + D * h + D])
    W_stack = np.concatenate(rows, 0)        # [512, 1024]
    w_qk = np.ascontiguousarray(W_stack.T.reshape(8, P, 512), BF)
    # only the q biases (first two 128-row groups); k bias is dropped
    # (it shifts every score in a row by the same amount -> softmax-inv).
    b_q = np.ascontiguousarray(
        np.concatenate(brows[:4], 0).reshape(2, P).T, np.float32)

    W_v4 = w_qkv[2 * C + D * h0: 2 * C + D * h0 + 256]
    w_v = np.ascontiguousarray(W_v4.T.reshape(8, P, 256), BF)

    w_p = np.ascontiguousarray(w_proj.T.reshape(8, P, 1024), BF)
    # effective proj bias per head: b_proj + w_proj @ tile(bv_h, 16)
    b_p = np.empty((4, P, 1024), np.float32)
    for hh in range(4):
        bv = b_qkv[2 * C + D * (h0 + hh): 2 * C + D * (h0 + hh) + D]
        eff = b_proj + w_proj @ np.tile(bv, 16)
        b_p[hh] = np.broadcast_to(eff, (P, 1024))
    b_p = np.ascontiguousarray(b_p, BF)
    tri = (np.arange(P)[None, :] >= np.arange(P)[:, None]).astype(BF)
    tri2 = np.ascontiguousarray(
        np.broadcast_to(tri[:, None, :], (P, 2, P)), BF)
    return {"xT": xT, "w_qk": w_qk, "w_v": w_v, "b_q": b_q,
            "w_p": w_p, "b_p": b_p, "tri2": tri2}


def _is_causal(mask: np.ndarray) -> bool:
    if mask.shape != (B, N, N):
        return False
    tril = np.tril(np.ones((N, N), bool))
    return bool(all(np.array_equal(mask[i], tril) for i in range(mask.shape[0])))


def _numpy_fallback(x, attention_mask, w_qkv, b_qkv, w_proj, b_proj):
    b, n, c = x.shape
    qkv = x @ w_qkv.T + b_qkv
    qkv = qkv.reshape(b, n, 3, H, D).transpose(2, 0, 3, 1, 4)
    q, k, v = qkv[0], qkv[1], qkv[2]
    dots = np.einsum("bhid,bhjd->bhij", q, k) * SCALE
    mask_value = -np.finfo(dots.dtype).max
    dots = np.where(attention_mask[:, None, :, :], dots, mask_value)
    dots = dots - dots.max(axis=-1, keepdims=True)
    e = np.exp(dots)
    attn = e / e.sum(axis=-1, keepdims=True)
    out = np.einsum("bhij,bhjd->bhid", attn, v)
    out = out.reshape(b, n, c)
    return (out @ w_proj.T + b_proj).astype(np.float32)


def kernel(**inputs) -> np.ndarray:
    x = np.asarray(inputs["x"], np.float32)
    mask = np.asarray(inputs["attention_mask"])
    w_qkv = np.asarray(inputs["w_qkv"], np.float32)
    b_qkv = np.asarray(inputs["b_qkv"], np.float32)
    w_proj = np.asarray(inputs["w_proj"], np.float32)
    b_proj = np.asarray(inputs["b_proj"], np.float32)

    if not _is_causal(mask):
        return _numpy_fallback(x, mask, w_qkv, b_qkv, w_proj, b_proj)

    nc = build_bass()
    in_maps = [_core_inputs(c, x, w_qkv, b_qkv, w_proj, b_proj)
               for c in range(8)]
    res = run_bass_kernel_spmd(nc, in_maps, core_ids=list(range(8)))
    out = np.empty((B, N, C), np.float32)
    for c in range(8):
        b = c // 4
        h0 = 4 * (c % 4)
        out[b, P * h0:P * h0 + 512, :] = res.results[c]["out"]
    return out
